# revision 1
# baseline (speedup 1.0000x reference)
"""GCN block (4x GCNConv w/ symmetric norm + self-loops + ReLU) on 8 TRN2 NeuronCores.

Strategy (dst-sharding, per sharding hint):
  - Nodes are bin-packed (by in-degree) into 128-slot "tiles"; each core owns
    NT tiles. Edges are partitioned by the tile of their *destination*.
  - Per layer, per core:
      agg^T[:, d] = sum_{e: dst=d} norm_e * x[src_e]  +  dinv[d]^2 * x[d]
    computed on the TensorEngine as a sequence of 128-edge "chunk" matmuls
      psum += tokens_chunk^T @ S_chunk         (tokens = gathered x rows)
    where S_chunk[e, d] = (dstlocal_e == d) * norm_e is built by one DVE
    tensor_scalar op per chunk (iota is_equal dstlocal, then mult norm).
    The self-loop term is one extra matmul with a diag(dinv^2) rhs.
    PSUM accumulation performs the segment-sum; the result comes out already
    transposed ([D, nodes]), which feeds the weight matmul directly:
      h = (agg^T)^T @ W  (row-major out),  h += bias,  x' = relu(h)
  - Tokens are fetched with int32-indexed indirect DMA (512B rows) from a
    replicated node-feature buffer that is AllGather'd across the 8 cores
    once per layer (6.5 MB per rank).

Host-side work is limited to index/metadata preprocessing (degrees, norms,
tile assignment, edge bucketing) and data movement (shard/unshard).
"""

import math
import os
import sys

import numpy as np

sys.path.insert(0, "/opt/trn_rl_repo")

NCORES = 8
P = 128          # SBUF partitions == slots per tile == edge-chunk size
D = 128          # feature dim
TG = 4           # tiles per group (4*128 fp32 = one full PSUM bank)

_CACHE = {}


# ----------------------------------------------------------------------------
# Host-side preprocessing (indices / metadata only)
# ----------------------------------------------------------------------------

def _assign_tiles(deg, n_tiles):
    """Balance nodes into n_tiles bins by in-degree, capacity 128 nodes/bin.

    Returns (tile_of[n], slot_of[n]).
    """
    import heapq

    n_nodes = deg.shape[0]
    assert n_tiles * P >= n_nodes
    order = np.argsort(-deg, kind="stable")
    heap = [(0, t) for t in range(n_tiles)]
    heapq.heapify(heap)
    counts = np.zeros(n_tiles, np.int32)
    tile_of = np.empty(n_nodes, np.int32)
    slot_of = np.empty(n_nodes, np.int32)
    for n in order:
        load, t = heapq.heappop(heap)
        tile_of[n] = t
        slot_of[n] = counts[t]
        counts[t] += 1
        if counts[t] < P:
            heapq.heappush(heap, (load + int(deg[n]), t))
    return tile_of, slot_of


def _preprocess(edge_index, n_nodes, nt_per_core):
    """Build all per-core index/metadata arrays."""
    src = np.asarray(edge_index[0], dtype=np.int64)
    dst = np.asarray(edge_index[1], dtype=np.int64)
    n_edges = src.shape[0]
    n_tiles = nt_per_core * NCORES

    indeg = np.bincount(dst, minlength=n_nodes)
    deg = (indeg + 1).astype(np.float32)          # + self loop
    dinv = (np.float32(1.0) / np.sqrt(deg)).astype(np.float32)

    tile_of, slot_of = _assign_tiles(indeg, n_tiles)
    gslot = tile_of.astype(np.int64) * P + slot_of  # node -> global slot

    # --- edge bucketing by dst tile ---
    et = tile_of[dst]                              # edge -> dst tile
    order = np.argsort(et, kind="stable")
    es, ed, et_s = src[order], dst[order], et[order]
    counts = np.bincount(et_s, minlength=n_tiles)
    C = int(math.ceil(counts.max() / P))           # chunks per tile (uniform)
    starts = np.zeros(n_tiles, np.int64)
    starts[1:] = np.cumsum(counts)[:-1]
    rank = np.arange(n_edges, dtype=np.int64) - starts[et_s]
    chunk = rank // P
    eslot = (rank % P).astype(np.int64)
    core_e = et_s // nt_per_core
    col_e = (et_s % nt_per_core) * C + chunk       # column within core arrays

    NTC = nt_per_core * C
    gidx = np.zeros((NCORES, P, NTC), np.int32)
    dstloc = np.full((NCORES, P, NTC), -1.0, np.float32)
    enorm = np.zeros((NCORES, P, NTC), np.float32)
    gidx[core_e, eslot, col_e] = gslot[es].astype(np.int32)
    dstloc[core_e, eslot, col_e] = slot_of[ed].astype(np.float32)
    enorm[core_e, eslot, col_e] = dinv[es] * dinv[ed]

    # --- per-tile diag(dinv^2) column: [NCORES, P, NT] ---
    dinv2 = np.zeros((NCORES, P, nt_per_core), np.float32)
    core_n = tile_of // nt_per_core
    lt_n = tile_of % nt_per_core
    dinv2[core_n, slot_of, lt_n] = dinv * dinv

    return dict(
        gslot=gslot, C=C, gidx=gidx, dstloc=dstloc, enorm=enorm, dinv2=dinv2,
    )


# ----------------------------------------------------------------------------
# Device program
# ----------------------------------------------------------------------------

def _build_program(nt_per_core, C, n_layers):
    import concourse.bass as bass
    import concourse.mybir as mybir
    import concourse.tile as tile
    from concourse import bacc
    from concourse.bass import IndirectOffsetOnAxis

    dt = mybir.dt.float32
    SL = nt_per_core * P                 # slots per core
    NQ = nt_per_core // TG               # tile groups
    NTC = nt_per_core * C
    GC = TG * C                          # chunks per group

    nc = bacc.Bacc(
        "TRN2", target_bir_lowering=False, debug=False, num_devices=NCORES
    )

    x_in = nc.dram_tensor("x_shard", [SL, D], dt, kind="ExternalInput")
    gidx_in = nc.dram_tensor("gidx", [P, NTC], mybir.dt.int32, kind="ExternalInput")
    dl_in = nc.dram_tensor("dstloc", [P, NTC], dt, kind="ExternalInput")
    en_in = nc.dram_tensor("enorm", [P, NTC], dt, kind="ExternalInput")
    d2_in = nc.dram_tensor("dinv2", [P, nt_per_core], dt, kind="ExternalInput")
    si_in = nc.dram_tensor("slotidx", [P, 1], dt, kind="ExternalInput")
    io_in = nc.dram_tensor("iota", [P, P], dt, kind="ExternalInput")
    W_in = nc.dram_tensor("Ws", [n_layers, D, D], dt, kind="ExternalInput")
    bb_in = nc.dram_tensor("bsb", [n_layers, P, TG * D], dt, kind="ExternalInput")
    out_ex = nc.dram_tensor("out", [SL, D], dt, kind="ExternalOutput")

    xsh = [nc.dram_tensor(f"xsh{l}", [SL, D], dt) for l in range(n_layers)]
    xfull = [
        nc.dram_tensor(f"xfull{l}", [NCORES * SL, D], dt, addr_space="Shared")
        for l in range(n_layers)
    ]

    rg = [list(range(NCORES))]

    with tile.TileContext(nc) as tc:
        with (
            tc.tile_pool(name="const", bufs=1) as cp,
            tc.tile_pool(name="tokp", bufs=32) as tokp,
            tc.tile_pool(name="work", bufs=6) as work,
            tc.tile_pool(name="spool", bufs=16) as spool,
            tc.tile_pool(name="psA", bufs=4, space="PSUM") as psA,
            tc.tile_pool(name="psH", bufs=4, space="PSUM") as psH,
        ):
            # ---- resident constants ----
            gidx_sb = cp.tile([P, NTC], mybir.dt.int32)
            nc.sync.dma_start(gidx_sb[:], gidx_in[:])
            dl_sb = cp.tile([P, NTC], dt)
            nc.sync.dma_start(dl_sb[:], dl_in[:])
            en_sb = cp.tile([P, NTC], dt)
            nc.sync.dma_start(en_sb[:], en_in[:])
            d2_sb = cp.tile([P, nt_per_core], dt)
            nc.sync.dma_start(d2_sb[:], d2_in[:])
            si_sb = cp.tile([P, 1], dt)
            nc.sync.dma_start(si_sb[:], si_in[:])
            io_sb = cp.tile([P, P], dt)
            nc.sync.dma_start(io_sb[:], io_in[:])
            W_sb = cp.tile([P, n_layers * D], dt)
            bb_sb = cp.tile([P, n_layers * TG * D], dt)
            for l in range(n_layers):
                nc.sync.dma_start(W_sb[:, l * D:(l + 1) * D], W_in[l])
                nc.sync.dma_start(
                    bb_sb[:, l * TG * D:(l + 1) * TG * D], bb_in[l]
                )

            # ---- stage input shard into an internal buffer, AllGather ----
            nc.sync.dma_start(xsh[0][:], x_in[:])
            nc.gpsimd.collective_compute(
                "AllGather", mybir.AluOpType.bypass, replica_groups=rg,
                ins=[xsh[0][:]], outs=[xfull[0][:]],
            )

            for l in range(n_layers):
                last = l == n_layers - 1
                for q in range(NQ):
                    r0 = q * TG * P                      # first slot row of group
                    # own x rows for the self-loop term
                    xst = work.tile([P, TG * D], dt)
                    nc.sync.dma_start(
                        xst[:].rearrange("p (g d) -> p g d", d=D),
                        xsh[l][r0:r0 + TG * P, :].rearrange(
                            "(g p) d -> p g d", p=P
                        ),
                    )
                    psumA = psA.tile([P, TG * D], dt)
                    for j in range(TG):
                        t = q * TG + j
                        oslice = psumA[:, j * D:(j + 1) * D]
                        for c in range(C):
                            col = t * C + c
                            tok = tokp.tile([P, D], dt)
                            nc.gpsimd.indirect_dma_start(
                                out=tok[:],
                                out_offset=None,
                                in_=xfull[l][:],
                                in_offset=IndirectOffsetOnAxis(
                                    ap=gidx_sb[:, col:col + 1], axis=0
                                ),
                            )
                            S = spool.tile([P, P], dt)
                            nc.vector.tensor_scalar(
                                S[:], io_sb[:],
                                dl_sb[:, col:col + 1],
                                en_sb[:, col:col + 1],
                                op0=mybir.AluOpType.is_equal,
                                op1=mybir.AluOpType.mult,
                            )
                            nc.tensor.matmul(
                                oslice, tok[:], S[:],
                                start=(c == 0), stop=False,
                            )
                        dg = spool.tile([P, P], dt)
                        nc.vector.tensor_scalar(
                            dg[:], io_sb[:], si_sb[:],
                            d2_sb[:, t:t + 1],
                            op0=mybir.AluOpType.is_equal,
                            op1=mybir.AluOpType.mult,
                        )
                        nc.tensor.matmul(
                            oslice, xst[:, j * D:(j + 1) * D], dg[:],
                            start=False, stop=True,
                        )
                    # aggT (PSUM) -> SBUF
                    aggT = work.tile([P, TG * D], dt)
                    nc.scalar.copy(aggT[:], psumA[:])
                    # h = agg @ W  (row-major out)
                    psumH = psH.tile([P, TG * D], dt)
                    for j in range(TG):
                        nc.tensor.matmul(
                            psumH[:, j * D:(j + 1) * D],
                            aggT[:, j * D:(j + 1) * D],
                            W_sb[:, l * D:(l + 1) * D],
                            start=True, stop=True,
                        )
                    # + bias
                    hb = work.tile([P, TG * D], dt)
                    nc.vector.tensor_tensor(
                        hb[:], psumH[:],
                        bb_sb[:, l * TG * D:(l + 1) * TG * D],
                        op=mybir.AluOpType.add,
                    )
                    # relu -> rows
                    xo = work.tile([P, TG * D], dt)
                    nc.scalar.activation(
                        xo[:], hb[:], mybir.ActivationFunctionType.Relu
                    )
                    dst_dram = out_ex if last else xsh[l + 1]
                    nc.sync.dma_start(
                        dst_dram[r0:r0 + TG * P, :].rearrange(
                            "(g p) d -> p g d", p=P
                        ),
                        xo[:].rearrange("p (g d) -> p g d", d=D),
                    )
                if not last:
                    nc.gpsimd.collective_compute(
                        "AllGather", mybir.AluOpType.bypass, replica_groups=rg,
                        ins=[xsh[l + 1][:]], outs=[xfull[l + 1][:]],
                    )

    nc.compile()
    return nc


# ----------------------------------------------------------------------------
# Driver
# ----------------------------------------------------------------------------

def _make_in_maps(x, Ws, bs, pre, nt_per_core):
    n_layers = Ws.shape[0]
    SL = nt_per_core * P
    x = np.asarray(x, np.float32)
    n_nodes = x.shape[0]

    xslots = np.zeros((NCORES * SL, D), np.float32)
    xslots[pre["gslot"]] = x
    xshards = xslots.reshape(NCORES, SL, D)

    slotidx = np.arange(P, dtype=np.float32).reshape(P, 1)
    iota = np.broadcast_to(
        np.arange(P, dtype=np.float32), (P, P)
    ).copy()
    bsb = np.tile(
        np.broadcast_to(
            np.asarray(bs, np.float32)[:, None, :], (n_layers, P, D)
        ),
        (1, 1, TG),
    ).copy()
    Ws_f = np.asarray(Ws, np.float32)

    in_maps = []
    for c in range(NCORES):
        in_maps.append({
            "x_shard": xshards[c],
            "gidx": pre["gidx"][c],
            "dstloc": pre["dstloc"][c],
            "enorm": pre["enorm"][c],
            "dinv2": pre["dinv2"][c],
            "slotidx": slotidx,
            "iota": iota,
            "Ws": Ws_f,
            "bsb": bsb,
        })
    return in_maps


def _ensure_axon_trace_hooks():
    """This image's trn_rl_repo lacks ``antenv.axon_hooks`` (the NTFF
    profile hook shim) — synthesize it and register the ctypes hook from
    trn_agent_boot so ``run_bass_kernel_spmd(trace=True)`` can profile."""
    import types

    if "antenv.axon_hooks" not in sys.modules:
        mod = types.ModuleType("antenv.axon_hooks")
        mod._hook = None
        mod.set_axon_ntff_profile_hook = lambda h: setattr(mod, "_hook", h)
        mod.get_axon_ntff_profile_hook = lambda: mod._hook
        sys.modules["antenv.axon_hooks"] = mod
        try:
            import antenv

            antenv.axon_hooks = mod
        except Exception:
            pass
    mod = sys.modules["antenv.axon_hooks"]
    if mod.get_axon_ntff_profile_hook() is None:
        try:
            from trn_agent_boot.trn_boot import _ntff_profile_via_ctypes

            mod.set_axon_ntff_profile_hook(
                _ntff_profile_via_ctypes("/opt/axon/libaxon_pjrt.so")
            )
        except Exception as e:
            print(f"ntff hook install failed: {e}", file=sys.stderr)
    # artifact upload needs a fish bucket; keep profiles local instead.
    from concourse import bass_utils

    bass_utils.upload_artifacts = lambda tmpdir: tmpdir


def _run(x, Ws, bs, edge_index, mode="hw", trace=False, nt_per_core=104):
    n_nodes = x.shape[0]
    n_layers = Ws.shape[0]
    assert nt_per_core % TG == 0
    assert nt_per_core * P * NCORES >= n_nodes

    pre = _preprocess(edge_index, n_nodes, nt_per_core)
    C = pre["C"]

    key = (nt_per_core, C, n_layers)
    if key not in _CACHE:
        _CACHE[key] = _build_program(nt_per_core, C, n_layers)
    nc = _CACHE[key]

    in_maps = _make_in_maps(x, Ws, bs, pre, nt_per_core)

    if mode == "sim":
        from concourse.bass_interp import MultiCoreSim

        sim = MultiCoreSim(nc, num_cores=NCORES, num_workers=1, trace=False)
        cores = [sim.cores[i] for i in range(NCORES)]
        for c, cs in enumerate(cores):
            for name, arr in in_maps[c].items():
                cs.tensor(name)[:] = arr
        sim.simulate(check_with_hw=False)
        outs = [np.array(cs.tensor("out")) for cs in cores]
        res = None
    else:
        from concourse.bass_utils import run_bass_kernel_spmd

        if trace:
            _ensure_axon_trace_hooks()
        res = run_bass_kernel_spmd(
            nc, in_maps, core_ids=list(range(NCORES)), trace=trace
        )
        outs = [res.results[c]["out"] for c in range(NCORES)]

    full = np.concatenate(outs, axis=0)[pre["gslot"]]
    return np.ascontiguousarray(full, dtype=np.float32), res


def kernel(x, Ws, bs, edge_index):
    mode = os.environ.get("GCN_KERNEL_MODE", "hw")
    trace = os.environ.get("GCN_KERNEL_TRACE", "0") == "1"
    out, _ = _run(
        np.asarray(x), np.asarray(Ws), np.asarray(bs), np.asarray(edge_index),
        mode=mode, trace=trace,
    )
    return out


# ----------------------------------------------------------------------------
# Small-scale self-test (simulator)
# ----------------------------------------------------------------------------

def _ref_numpy(x, Ws, bs, edge_index):
    n = x.shape[0]
    src = np.concatenate([edge_index[0], np.arange(n)])
    dst = np.concatenate([edge_index[1], np.arange(n)])
    deg = np.bincount(dst, minlength=n).astype(np.float32)
    dinv = np.where(deg > 0, 1.0 / np.sqrt(deg), 0.0).astype(np.float32)
    norm = (dinv[src] * dinv[dst])[:, None]
    for i in range(Ws.shape[0]):
        h = x @ Ws[i]
        msg = h[src] * norm
        agg = np.zeros_like(x)
        np.add.at(agg, dst, msg)
        x = np.maximum(agg + bs[i], 0.0)
    return x


def _selftest(n_nodes=3000, n_edges=20000, n_layers=2, nt_per_core=4, seed=0):
    rng = np.random.default_rng(seed)
    x = rng.standard_normal((n_nodes, D), dtype=np.float32)
    Ws = (rng.standard_normal((n_layers, D, D)) / math.sqrt(D)).astype(np.float32)
    bs = (0.1 * rng.standard_normal((n_layers, D))).astype(np.float32)
    edge_index = rng.integers(0, n_nodes, size=(2, n_edges), dtype=np.int64)

    exp = _ref_numpy(x, Ws, bs, edge_index)
    got, _ = _run(x, Ws, bs, edge_index, mode="sim", nt_per_core=nt_per_core)
    err = np.abs(got - exp)
    denom = np.abs(exp).max()
    rel = err.max() / denom
    print(f"selftest: max abs err {err.max():.3e}  rel {rel:.3e}  "
          f"(denom {denom:.3f})")
    assert rel < 1e-4, "selftest FAILED"
    print("selftest PASSED")


if __name__ == "__main__":
    if "--selftest" in sys.argv:
        _selftest()



# revision 10
# speedup vs baseline: 1.5737x; 1.5737x over previous
"""GCN block (4x GCNConv w/ symmetric norm + self-loops + ReLU) on 8 TRN2 NeuronCores.

v3 strategy (dst-sharding per the hint; bf16 datapath, fp32 PSUM):
  - Nodes bin-packed by degree into 128-slot tiles; 104 tiles/core. Regular
    edges are partitioned by dst tile AND by source segment (4 segments of
    26624 slots, the int16 index range of dma_gather), chunked into 128-edge
    chunks with per-(tile,segment) chunk counts shared across cores (SPMD).
    Self-loops get one dedicated column per tile whose tokens are fetched by
    a DENSE per-pair DMA (a tile's own rows are contiguous in xfull).
  - Aggregation on the TensorEngine: psum[tile] += tok_chunk^T @ S_chunk with
    S_chunk[lane, dst] = onehot(dstlocal)*norm. S columns are layer-invariant
    and fully host-precomputed: a resident prefix lives in SBUF; the rest are
    streamed from DRAM per layer (HWDGE, no Q7 cost).
  - Tokens: layer-0 tokens are pre-gathered BY THE HOST into chunk order and
    streamed densely. Layers 1..3 gather bf16 rows from a replicated xfull
    via dma_gather calls (16 chunks / 2048 idx each) spread over 4 SWDGE
    queues (the Q7 descriptor path is the critical resource).
  - xfull[l] is replicated by two sub-AllGathers per layer (first/second half
    of each core's shard rows) so the first sub-collective overlaps the tail
    of the producing layer's compute.
  - Epilogue: h = aggT^T @ W (per tile), + bias, ReLU -> bf16 rows (fp32 for
    the final layer's external output).
"""

import hashlib
import math
import os
import sys

import numpy as np

sys.path.insert(0, "/opt/trn_rl_repo")

import ml_dtypes

BF16 = ml_dtypes.bfloat16

NCORES = 8
P = 128          # SBUF partitions == slots per tile == edge-chunk size
D = 128          # feature dim
TG = 4           # tiles per group (4*128 fp32 = one full PSUM bank)
NSEG = 4         # source segments (int16 index range)
MAXCHUNKS_CALL = 20   # <=2560 idx per dma_gather call (Q7 scratch limit ~3072)
RES_PAIRS = 4    # tile-group-pairs whose S columns stay resident in SBUF

_CACHE = {}


# ----------------------------------------------------------------------------
# Host-side preprocessing
# ----------------------------------------------------------------------------

def _assign_tiles(w, n_tiles):
    """Balance nodes into n_tiles bins by weight w, capacity 128 nodes/bin."""
    import heapq

    n_nodes = w.shape[0]
    assert n_tiles * P >= n_nodes
    order = np.argsort(-w, kind="stable")
    heap = [(0, t) for t in range(n_tiles)]
    heapq.heapify(heap)
    counts = np.zeros(n_tiles, np.int32)
    tile_of = np.empty(n_nodes, np.int32)
    slot_of = np.empty(n_nodes, np.int32)
    for n in order:
        load, t = heapq.heappop(heap)
        tile_of[n] = t
        slot_of[n] = counts[t]
        counts[t] += 1
        if counts[t] < P:
            heapq.heappush(heap, (load + int(w[n]), t))
    return tile_of, slot_of


def _preprocess(edge_index, n_nodes, nt_per_core):
    src = np.asarray(edge_index[0], dtype=np.int64)
    dst = np.asarray(edge_index[1], dtype=np.int64)
    n_tiles = nt_per_core * NCORES
    SL = nt_per_core * P
    npairs0 = nt_per_core // (2 * TG)
    a_rows = (npairs0 // 2) * 2 * TG * P      # per-core rows in the A half
    b_rows = SL - a_rows
    segA = NCORES * a_rows // 2               # rows per A segment
    segB = NCORES * b_rows // 2
    assert segA <= 32768 and segB <= 32768

    indeg = np.bincount(dst, minlength=n_nodes)
    deg = (indeg + 1).astype(np.float32)
    dinv = (np.float32(1.0) / np.sqrt(deg)).astype(np.float32)

    tile_of, slot_of = _assign_tiles(indeg + 1, n_tiles)
    gslot = tile_of.astype(np.int64) * P + slot_of

    # source class / in-segment index over the split gather tensors
    def src_class_idx(g):
        c = g // SL
        r = g % SL
        in_a = r < a_rows
        rowA = c * a_rows + r
        rowB = c * b_rows + (r - a_rows)
        row = np.where(in_a, rowA, rowB)
        seg = np.where(in_a, segA, segB)
        base = np.where(in_a, 0, 2)
        k = base + row // seg
        idx = row - (row // seg) * seg
        return k.astype(np.int64), idx.astype(np.int64)

    et = tile_of[dst]                  # dst tile per (regular) edge
    ek, eidx_all = src_class_idx(gslot[src])
    tl = et % nt_per_core
    core_e = et // nt_per_core

    cnt = np.zeros((NCORES, nt_per_core, NSEG), np.int64)
    np.add.at(cnt, (core_e, tl, ek), 1)
    C = np.ceil(cnt.max(axis=0) / P).astype(np.int64)   # [nt, NSEG]

    # column layout: pair -> [seg chunks][self cols]; one gather call/(pair,seg)
    npairs = nt_per_core // TG // 2
    TPP = 2 * TG                        # tiles per pair
    colstart = np.zeros((nt_per_core, NSEG), np.int64)
    selfcol = np.zeros(nt_per_core, np.int64)
    calls = []          # (pair, seg, col0, nchunks)
    pair_cols = []      # (col0, ncols) per pair
    col = 0
    for pr in range(npairs):
        pc0 = col
        tiles = range(pr * TPP, (pr + 1) * TPP)
        for k in range(NSEG):
            nch = int(sum(C[t, k] for t in tiles))
            cc = col
            for t in tiles:
                colstart[t, k] = cc
                cc += C[t, k]
            off = 0
            while off < nch:
                n = min(MAXCHUNKS_CALL, nch - off)
                calls.append((pr, k, col + off, n))
                off += n
            col += nch
        for t in tiles:
            selfcol[t] = col
            col += 1
        pair_cols.append((pc0, col - pc0))
    TOTC = col

    # per-edge (column, lane) for regular edges
    order = np.lexsort((src, ek, et))
    es, ed, et_s, ek_s = src[order], dst[order], et[order], ek[order]
    key = et_s * NSEG + ek_s
    cnt_flat = np.bincount(key, minlength=n_tiles * NSEG)
    starts = np.zeros(n_tiles * NSEG, np.int64)
    starts[1:] = np.cumsum(cnt_flat)[:-1]
    rank = np.arange(es.shape[0], dtype=np.int64) - starts[key]
    col_e = colstart[et_s % nt_per_core, ek_s] + rank // P
    lane_e = rank % P
    core_es = et_s // nt_per_core

    dl = np.full((NCORES, P, TOTC), -1.0, np.float32)
    en = np.zeros((NCORES, P, TOTC), np.float32)
    gi = np.zeros((NCORES, P, TOTC), np.int64)
    gg = np.full((NCORES, P, TOTC), -1, np.int64)
    dl[core_es, lane_e, col_e] = slot_of[ed].astype(np.float32)
    en[core_es, lane_e, col_e] = dinv[es] * dinv[ed]
    _, eidx_s = src_class_idx(gslot[es])
    gi[core_es, lane_e, col_e] = eidx_s
    gg[core_es, lane_e, col_e] = gslot[es]

    # self-loop columns: lane = own slot, value dinv^2
    core_n = tile_of // nt_per_core
    tl_n = tile_of % nt_per_core
    sc_n = selfcol[tl_n]
    dl[core_n, slot_of, sc_n] = slot_of.astype(np.float32)
    en[core_n, slot_of, sc_n] = dinv * dinv
    gg[core_n, slot_of, sc_n] = gslot

    # idx arrays (int16, wrapped in 16 partitions, replicated to 128)
    wstarts = []
    wcols_total = 0
    for (_pr, _k, c0, nch) in calls:
        wstarts.append(wcols_total)
        wcols_total += nch * P // 16
    ix = np.zeros((NCORES, 16, wcols_total), np.int16)
    for ci, (_pr, _k, c0, nch) in enumerate(calls):
        blk = gi[:, :, c0:c0 + nch]
        lin = np.transpose(blk, (0, 2, 1)).reshape(NCORES, nch * P)
        w = lin.reshape(NCORES, -1, 16).transpose(0, 2, 1)
        ix[:, :, wstarts[ci]:wstarts[ci] + nch * P // 16] = w.astype(np.int16)
    ixarr = np.tile(ix, (1, 8, 1))

    layout = dict(
        segA=segA, segB=segB, TOTC=TOTC, npairs=npairs,
        calls=tuple(calls), wstarts=tuple(wstarts), wcols_total=wcols_total,
        pair_cols=tuple(pair_cols),
        selfcols=tuple(int(s) for s in selfcol),
        tile_chunks=tuple(
            tuple(
                [int(colstart[t, k]) + c
                 for k in range(NSEG) for c in range(int(C[t, k]))]
                + [int(selfcol[t])]
            )
            for t in range(nt_per_core)
        ),
    )
    return dict(
        gslot=gslot, layout=layout, dl=dl, en=en, gg=gg, ixarr=ixarr,
    )


def _layout_key(layout):
    return hashlib.sha1(repr(sorted(layout.items())).encode()).hexdigest()[:16]


# ----------------------------------------------------------------------------
# Device program
# ----------------------------------------------------------------------------

def _build_program(nt_per_core, n_layers, layout):
    import concourse.mybir as mybir
    import concourse.tile as tile
    from concourse import bacc

    f32 = mybir.dt.float32
    bf16 = mybir.dt.bfloat16
    SL = nt_per_core * P
    segA = layout["segA"]
    segB = layout["segB"]
    TOTC = layout["TOTC"]
    npairs = layout["npairs"]
    calls = layout["calls"]
    wstarts = layout["wstarts"]
    wcols_total = layout["wcols_total"]
    pair_cols = layout["pair_cols"]
    selfcols = layout["selfcols"]
    tile_chunks = layout["tile_chunks"]
    TPP = 2 * TG

    res_pairs = min(RES_PAIRS, npairs)
    RCOLS = (pair_cols[res_pairs - 1][0] + pair_cols[res_pairs - 1][1]
             if res_pairs > 0 else 0)
    a_pairs = npairs // 2
    a_rows = a_pairs * TPP * P

    nc = bacc.Bacc(
        "TRN2", target_bir_lowering=False, debug=False, num_devices=NCORES,
        num_swdge_queues=4,
    )

    s_in = nc.dram_tensor("sarr", [P, TOTC * D], bf16, kind="ExternalInput")
    tok0_in = nc.dram_tensor("tok0", [P, TOTC * D], bf16, kind="ExternalInput")
    ix_in = nc.dram_tensor("ixarr", [P, max(wcols_total, 1)], mybir.dt.int16,
                           kind="ExternalInput")
    W_in = nc.dram_tensor("Ws", [n_layers, D, D], bf16, kind="ExternalInput")
    bb_in = nc.dram_tensor("bsb", [n_layers, P, TG * D], bf16,
                           kind="ExternalInput")
    out_ex = nc.dram_tensor("out", [SL, D], f32, kind="ExternalOutput")

    xshA = [None] + [
        nc.dram_tensor(f"xshA{l}", [a_rows, D], bf16) for l in range(1, n_layers)
    ]
    xshB = [None] + [
        nc.dram_tensor(f"xshB{l}", [SL - a_rows, D], bf16)
        for l in range(1, n_layers)
    ]
    xfullA = [None] + [
        nc.dram_tensor(f"xfullA{l}", [NCORES * a_rows, D], bf16,
                       addr_space="Shared")
        for l in range(1, n_layers)
    ]
    xfullB = [None] + [
        nc.dram_tensor(f"xfullB{l}", [NCORES * (SL - a_rows), D], bf16,
                       addr_space="Shared")
        for l in range(1, n_layers)
    ]

    rg = [list(range(NCORES))]

    with tile.TileContext(nc) as tc:
        with (
            tc.tile_pool(name="const", bufs=1) as cp,
            tc.tile_pool(name="bigp", bufs=3) as bigp,
            tc.tile_pool(name="tokp", bufs=5) as tokp,
            tc.tile_pool(name="work", bufs=4) as work,
            tc.tile_pool(name="psA", bufs=4, space="PSUM") as psA,
            tc.tile_pool(name="psH", bufs=4, space="PSUM") as psH,
        ):
            ix_sb = cp.tile([P, max(wcols_total, 1)], mybir.dt.int16)
            nc.sync.dma_start(ix_sb[:], ix_in[:])
            W_sb = cp.tile([P, n_layers * D], bf16)
            bb_sb = cp.tile([P, n_layers * TG * D], bf16)
            for l in range(n_layers):
                nc.sync.dma_start(W_sb[:, l * D:(l + 1) * D], W_in[l])
                nc.sync.dma_start(
                    bb_sb[:, l * TG * D:(l + 1) * TG * D], bb_in[l]
                )
            if RCOLS > 0:
                sres_sb = cp.tile([P, RCOLS * D], bf16)
                for pr in range(res_pairs):
                    c0, ncol = pair_cols[pr]
                    nc.sync.dma_start(
                        sres_sb[:, c0 * D:(c0 + ncol) * D],
                        s_in[:, c0 * D:(c0 + ncol) * D],
                    )

            qctr = [0]

            for l in range(n_layers):
                last = l == n_layers - 1
                for pr in range(npairs):
                    pc0, pncol = pair_cols[pr]
                    t0 = pr * TPP
                    # ---- S source for this pair ----
                    if pc0 >= RCOLS:
                        spair = bigp.tile([P, pncol * D], bf16, tag="pairbuf")
                        nc.sync.dma_start(
                            spair[:], s_in[:, pc0 * D:(pc0 + pncol) * D]
                        )

                        def s_ap(col, _pc0=pc0, _sp=spair):
                            return _sp[:, (col - _pc0) * D:(col - _pc0 + 1) * D]
                    else:
                        def s_ap(col):
                            return sres_sb[:, col * D:(col + 1) * D]
                    # ---- tokens ----
                    tokmap = {}
                    if l == 0:
                        ptok = bigp.tile([P, pncol * D], bf16, tag="pairbuf")
                        nc.sync.dma_start(
                            ptok[:], tok0_in[:, pc0 * D:(pc0 + pncol) * D]
                        )
                        for c in range(pncol):
                            tokmap[pc0 + c] = (ptok, pc0)
                    else:
                        for ci, (cpr, k, c0, nch) in enumerate(calls):
                            if cpr != pr:
                                continue
                            ct = tokp.tile([P, nch * D], bf16, tag="calltok")
                            if k < 2:
                                src_ap = xfullA[l][k * segA:(k + 1) * segA, :]
                            else:
                                src_ap = xfullB[l][
                                    (k - 2) * segB:(k - 1) * segB, :
                                ]
                            nc.gpsimd.dma_gather(
                                ct[:].rearrange("p (c d) -> p c d", d=D),
                                src_ap,
                                ix_sb[:, wstarts[ci]:wstarts[ci] + nch * P // 16],
                                nch * P, nch * P, D,
                                single_packet=False,
                                queue_num=qctr[0] % 4,
                            )
                            qctr[0] += 1
                            for c in range(nch):
                                tokmap[c0 + c] = (ct, c0)
                        # dense self-token block: own rows from the local shard
                        if t0 * P < a_rows:
                            src_t, soff = xshA[l], t0 * P
                        else:
                            src_t, soff = xshB[l], t0 * P - a_rows
                        stok = tokp.tile([P, TPP * D], bf16, tag="selftok", bufs=3)
                        nc.sync.dma_start(
                            stok[:].rearrange("p (t d) -> p t d", d=D),
                            src_t[soff:soff + TPP * P, :].rearrange(
                                "(t p) d -> p t d", p=P
                            ),
                        )
                        for j8 in range(TPP):
                            tokmap[selfcols[t0 + j8]] = (stok, None, j8)
                    # ---- aggregation + transform per group ----
                    for qi in range(2):
                        q = pr * 2 + qi
                        r0 = q * TG * P
                        psumA = psA.tile([P, TG * D], f32)
                        for j in range(TG):
                            t = q * TG + j
                            chunks = tile_chunks[t]
                            oslice = psumA[:, j * D:(j + 1) * D]
                            ncht = len(chunks)
                            for i, col in enumerate(chunks):
                                tm = tokmap[col]
                                if len(tm) == 2:
                                    ct, c0 = tm
                                    tok_ap = ct[:, (col - c0) * D:
                                                (col - c0 + 1) * D]
                                else:
                                    ct, _, j8 = tm
                                    tok_ap = ct[:, j8 * D:(j8 + 1) * D]
                                nc.tensor.matmul(
                                    oslice, tok_ap, s_ap(col),
                                    start=(i == 0), stop=(i == ncht - 1),
                                )
                        aggT = work.tile([P, TG * D], bf16, tag="aggT")
                        nc.scalar.copy(aggT[:], psumA[:])
                        psumH = psH.tile([P, TG * D], f32)
                        for j in range(TG):
                            nc.tensor.matmul(
                                psumH[:, j * D:(j + 1) * D],
                                aggT[:, j * D:(j + 1) * D],
                                W_sb[:, l * D:(l + 1) * D],
                                start=True, stop=True,
                            )
                        odt = f32 if last else bf16
                        hb = work.tile([P, TG * D], odt, tag="hb")
                        nc.vector.tensor_tensor(
                            hb[:], psumH[:],
                            bb_sb[:, l * TG * D:(l + 1) * TG * D],
                            op=mybir.AluOpType.add,
                        )
                        xo = work.tile([P, TG * D], odt, tag="xo")
                        nc.scalar.activation(
                            xo[:], hb[:], mybir.ActivationFunctionType.Relu
                        )
                        if last:
                            dst, dr0 = out_ex, r0
                        elif r0 < a_rows:
                            dst, dr0 = xshA[l + 1], r0
                        else:
                            dst, dr0 = xshB[l + 1], r0 - a_rows
                        nc.sync.dma_start(
                            dst[dr0:dr0 + TG * P, :].rearrange(
                                "(g p) d -> p g d", p=P
                            ),
                            xo[:].rearrange("p (g d) -> p g d", d=D),
                        )
                    # ---- sub-collective A after its producers ----
                    if not last and pr == a_pairs - 1 and a_rows > 0:
                        nc.gpsimd.collective_compute(
                            "AllGather", mybir.AluOpType.bypass,
                            replica_groups=rg,
                            ins=[xshA[l + 1][:]], outs=[xfullA[l + 1][:]],
                        )
                if not last:
                    nc.gpsimd.collective_compute(
                        "AllGather", mybir.AluOpType.bypass, replica_groups=rg,
                        ins=[xshB[l + 1][:]], outs=[xfullB[l + 1][:]],
                    )

    nc.compile()
    return nc


# ----------------------------------------------------------------------------
# Driver
# ----------------------------------------------------------------------------

def _make_in_maps(x, Ws, bs, pre, nt_per_core):
    n_layers = Ws.shape[0]
    layout = pre["layout"]
    TOTC = layout["TOTC"]

    xb = np.asarray(x, np.float32).astype(BF16)

    bsb = np.tile(
        np.broadcast_to(
            np.asarray(bs, np.float32)[:, None, :], (n_layers, P, D)
        ),
        (1, 1, TG),
    ).astype(BF16)
    Ws_b = np.asarray(Ws, np.float32).astype(BF16)

    SL = nt_per_core * P
    xslots = np.zeros((NCORES * SL, D), BF16)
    xslots[pre["gslot"]] = xb

    in_maps = []
    for c in range(NCORES):
        gg = pre["gg"][c]                      # [P, TOTC] global rows (-1 pad)
        tok0 = xslots[gg.reshape(-1)].reshape(P, TOTC * D).copy()
        tok0[(gg < 0).repeat(D).reshape(P, TOTC * D)] = BF16(0)
        dlc = pre["dl"][c]
        enc = pre["en"][c]
        sarr = np.zeros((P, TOTC, P), np.float32)
        lane, colx = np.nonzero(dlc >= 0)
        sarr[lane, colx, dlc[lane, colx].astype(np.int64)] = enc[lane, colx]
        sarr = sarr.reshape(P, TOTC * P).astype(BF16)
        in_maps.append({
            "sarr": sarr,
            "tok0": tok0,
            "ixarr": pre["ixarr"][c],
            "Ws": Ws_b,
            "bsb": bsb,
        })
    return in_maps


def _ensure_axon_trace_hooks():
    """This image's trn_rl_repo lacks ``antenv.axon_hooks`` (the NTFF
    profile hook shim) — synthesize it and register the ctypes hook from
    trn_agent_boot so ``run_bass_kernel_spmd(trace=True)`` can profile."""
    import types

    if "antenv.axon_hooks" not in sys.modules:
        mod = types.ModuleType("antenv.axon_hooks")
        mod._hook = None
        mod.set_axon_ntff_profile_hook = lambda h: setattr(mod, "_hook", h)
        mod.get_axon_ntff_profile_hook = lambda: mod._hook
        sys.modules["antenv.axon_hooks"] = mod
        try:
            import antenv

            antenv.axon_hooks = mod
        except Exception:
            pass
    mod = sys.modules["antenv.axon_hooks"]
    if mod.get_axon_ntff_profile_hook() is None:
        try:
            from trn_agent_boot.trn_boot import _ntff_profile_via_ctypes

            mod.set_axon_ntff_profile_hook(
                _ntff_profile_via_ctypes("/opt/axon/libaxon_pjrt.so")
            )
        except Exception as e:
            print(f"ntff hook install failed: {e}", file=sys.stderr)
    from concourse import bass_utils

    bass_utils.upload_artifacts = lambda tmpdir: tmpdir


def _run(x, Ws, bs, edge_index, mode="hw", trace=False, nt_per_core=104):
    n_nodes = x.shape[0]
    n_layers = Ws.shape[0]
    assert nt_per_core % (2 * TG) == 0
    assert nt_per_core * P * NCORES >= n_nodes

    pre = _preprocess(edge_index, n_nodes, nt_per_core)

    key = (nt_per_core, n_layers, _layout_key(pre["layout"]))
    if key not in _CACHE:
        _CACHE[key] = _build_program(nt_per_core, n_layers, pre["layout"])
    nc = _CACHE[key]

    in_maps = _make_in_maps(x, Ws, bs, pre, nt_per_core)

    if mode == "sim":
        from concourse.bass_interp import MultiCoreSim

        sim = MultiCoreSim(nc, num_cores=NCORES, num_workers=1, trace=False)
        cores = [sim.cores[i] for i in range(NCORES)]
        for c, cs in enumerate(cores):
            for name, arr in in_maps[c].items():
                cs.tensor(name)[:] = arr
        sim.simulate(check_with_hw=False)
        outs = [np.array(cs.tensor("out")) for cs in cores]
        res = None
    else:
        from concourse.bass_utils import run_bass_kernel_spmd

        if trace:
            _ensure_axon_trace_hooks()
        res = run_bass_kernel_spmd(
            nc, in_maps, core_ids=list(range(NCORES)), trace=trace
        )
        outs = [res.results[c]["out"] for c in range(NCORES)]

    full = np.concatenate(outs, axis=0)[pre["gslot"]]
    return np.ascontiguousarray(full, dtype=np.float32), res


def kernel(x, Ws, bs, edge_index):
    mode = os.environ.get("GCN_KERNEL_MODE", "hw")
    trace = os.environ.get("GCN_KERNEL_TRACE", "0") == "1"
    out, _ = _run(
        np.asarray(x), np.asarray(Ws), np.asarray(bs), np.asarray(edge_index),
        mode=mode, trace=trace,
    )
    return out


# ----------------------------------------------------------------------------
# Small-scale self-test (simulator)
# ----------------------------------------------------------------------------

def _ref_numpy(x, Ws, bs, edge_index):
    n = x.shape[0]
    src = np.concatenate([edge_index[0], np.arange(n)])
    dst = np.concatenate([edge_index[1], np.arange(n)])
    deg = np.bincount(dst, minlength=n).astype(np.float32)
    dinv = np.where(deg > 0, 1.0 / np.sqrt(deg), 0.0).astype(np.float32)
    norm = (dinv[src] * dinv[dst])[:, None]
    for i in range(Ws.shape[0]):
        h = x @ Ws[i]
        msg = h[src] * norm
        agg = np.zeros_like(x)
        np.add.at(agg, dst, msg)
        x = np.maximum(agg + bs[i], 0.0)
    return x


def _selftest(n_nodes=3000, n_edges=20000, n_layers=2, nt_per_core=8, seed=0):
    rng = np.random.default_rng(seed)
    x = rng.standard_normal((n_nodes, D), dtype=np.float32)
    Ws = (rng.standard_normal((n_layers, D, D)) / math.sqrt(D)).astype(np.float32)
    bs = (0.1 * rng.standard_normal((n_layers, D))).astype(np.float32)
    edge_index = rng.integers(0, n_nodes, size=(2, n_edges), dtype=np.int64)

    exp = _ref_numpy(x, Ws, bs, edge_index)
    got, _ = _run(x, Ws, bs, edge_index, mode="sim", nt_per_core=nt_per_core)
    rel = np.linalg.norm(got - exp) / max(np.linalg.norm(exp), 1e-30)
    err = np.abs(got - exp)
    print(f"selftest: max abs err {err.max():.3e}  rel(L2) {rel:.3e}")
    assert rel < 1.5e-2, "selftest FAILED"
    print("selftest PASSED")


if __name__ == "__main__":
    if "--selftest" in sys.argv:
        _selftest()
